# revision 11
# baseline (speedup 1.0000x reference)
"""Trainium2 Bass kernel for CoarseMatching (dual-softmax feature matching).

conf = softmax(sim, axis=2) * softmax(sim, axis=1),  sim = f0 @ f1^T / (C*TEMP)

Sharding: the L dimension of feature0 is split across 4 cores per batch
(2 batches x 4 chunks = 8 cores). Each core computes a [1280, 4800] row-slab
of conf (core 3 of each group is zero-padded from 960 to 1280 rows).

Per core (single NEFF, SPMD):
  sweep 1 (S-thirds outer so column-sum PSUM accumulators fit in 8 banks):
    sim tiles via fp32r matmul, E=exp(sim) with fused accum_out row-sums (rs),
    column-sums via ones-matmul accumulated in PSUM (partial: this core's rows).
  AllReduce(add) of cs partials across the 4-core batch group. The 320
  zero-pad rows contribute exactly exp(0)=1 each, removed via a -320 bias.
  sweep 2: recompute sim, conf = exp(2*sim - ln rs) * (1/cs) broadcast, DMA out.

mask / matched_conf: with randn inputs conf stays ~3 orders of magnitude below
THRESHOLD=0.2, so mask is all-False and matched_conf all-zero; the host checks
max(conf) and falls back to an exact numpy path if that ever fails.
"""

import numpy as np

TEMP = 0.1
THRESHOLD = 0.2
MARGIN = 2
N_BATCH = 2
L = 4800          # h0*w0
S = 4800          # h1*w1
C = 256
N_CORES = 8
LM = 1280         # padded per-core L-slab rows
PAD_ROWS = 4 * LM - L   # 320 zero rows per batch group (all on core 3)
SCALE = 1.0 / (C * TEMP)
CHUNK_STARTS = [0, 1280, 2560, 3840]

# S-thirds for sweep-1 column-sum PSUM accumulators: 4800 = 3 * 1600,
# each third split [1024, 576].
THIRDS = [(b, [(0, 1024), (1024, 576)]) for b in (0, 1600, 3200)]
# full-S chunking for sweep 2
S2_CHUNKS = [(0, 1024), (1024, 1024), (2048, 1024), (3072, 1024), (4096, 704)]
N_LTILES = LM // 128  # 10


def _build():
    from concourse import bacc, tile, mybir

    nc = bacc.Bacc(
        "TRN2", target_bir_lowering=False, debug=False, num_devices=N_CORES
    )
    f32 = mybir.dt.float32
    f32r = mybir.dt.float32r
    AF = mybir.ActivationFunctionType

    f0T_my = nc.dram_tensor("f0T_my", [C, LM], f32r, kind="ExternalInput").ap()
    f1T_full = nc.dram_tensor("f1T_full", [C, S], f32r, kind="ExternalInput").ap()
    onesw = nc.dram_tensor("onesw", [128, 128], f32r, kind="ExternalInput").ap()
    conf_out = nc.dram_tensor("conf_out", [LM, S], f32, kind="ExternalOutput").ap()

    with tile.TileContext(nc) as tc:
        with (
            tc.tile_pool(name="feat", bufs=1) as featp,
            tc.tile_pool(name="ep", bufs=3) as ep,
            tc.tile_pool(name="gp", bufs=3) as gp,
            tc.tile_pool(name="stats", bufs=1) as statp,
            tc.tile_pool(name="simps", bufs=2, space="PSUM") as simps,
            tc.tile_pool(name="csps", bufs=2, space="PSUM") as csps,
            tc.tile_pool(name="dram", bufs=1, space="DRAM") as dramp,
        ):
            # ---- load features (C on partitions, 2 K-halves side by side)
            sf0 = featp.tile([128, 2 * LM], f32r)
            sf1 = featp.tile([128, 2 * S], f32r)
            for k in range(2):
                nc.sync.dma_start(
                    sf0[:, k * LM:(k + 1) * LM], f0T_my[k * 128:(k + 1) * 128, :]
                )
                nc.sync.dma_start(
                    sf1[:, k * S:(k + 1) * S], f1T_full[k * 128:(k + 1) * 128, :]
                )

            ones = statp.tile([128, 128], f32r)
            nc.sync.dma_start(ones[:], onesw)

            rs_parts = statp.tile([128, N_LTILES * 6], f32)
            cs_row = statp.tile([1, S], f32)

            # ================= sweep 1: stats =================
            pending_cs = []  # deferred ones-matmuls so PE never waits on ACT

            def flush_one_cs():
                if pending_cs:
                    args = pending_cs.pop(0)
                    for a in args:
                        nc.tensor.matmul(*a[0], **a[1])

            for third, (tbase, tchunks) in enumerate(THIRDS):
                cs_acc = [
                    csps.tile([128, w], f32, name=f"cs_{tbase}_{ci}", tag="csacc")
                    for ci, (off, w) in enumerate(tchunks)
                ]
                for i in range(N_LTILES):
                    for ci, (off, w) in enumerate(tchunks):
                        ps = simps.tile([128, 1024], f32, tag="simps")
                        for k in range(2):
                            for so in range(0, w, 512):
                                sw = min(512, w - so)
                                nc.tensor.matmul(
                                    ps[:, so:so + sw],
                                    sf0[:, k * LM + i * 128: k * LM + (i + 1) * 128],
                                    sf1[:, k * S + tbase + off + so:
                                        k * S + tbase + off + so + sw],
                                    start=(k == 0),
                                    stop=(k == 1),
                                )
                        e = ep.tile([128, 1024], f32r, tag="etile")
                        # column layout must match the (i, c) grouping of the
                        # tensor_reduce below: col = i*6 + (third*2 + ci)
                        part = i * 6 + third * 2 + ci
                        nc.scalar.activation(
                            e[:, :w], ps[:, :w], AF.Exp, scale=SCALE,
                            accum_out=rs_parts[:, part:part + 1],
                        )
                        mm_args = []
                        for so in range(0, w, 512):
                            sw = min(512, w - so)
                            mm_args.append((
                                (
                                    cs_acc[ci][:, so:so + sw],
                                    ones[:],
                                    e[:, so:so + sw],
                                ),
                                dict(start=(i == 0), stop=(i == N_LTILES - 1)),
                            ))
                        pending_cs.append(mm_args)
                        if len(pending_cs) > 1:
                            flush_one_cs()
                while pending_cs:
                    flush_one_cs()
                for ci, (off, w) in enumerate(tchunks):
                    nc.vector.tensor_copy(
                        cs_row[0:1, tbase + off:tbase + off + w],
                        cs_acc[ci][0:1, :w],
                    )

            # ---- rs -> -ln(rs)  [128, N_LTILES]
            rs = statp.tile([128, N_LTILES], f32)
            nc.vector.tensor_reduce(
                rs[:],
                rs_parts[:].rearrange("p (i c) -> p i c", c=6),
                axis=mybir.AxisListType.X,
                op=mybir.AluOpType.add,
            )
            neg_lnrs = statp.tile([128, N_LTILES], f32)
            nc.scalar.activation(neg_lnrs[:], rs[:], AF.Ln)
            nc.vector.tensor_scalar_mul(neg_lnrs[:], neg_lnrs[:], -1.0)

            # ---- AllReduce cs partials across the 4-core batch group
            cc_in = dramp.tile([1, S], f32)
            cc_out = dramp.tile([1, S], f32)
            nc.sync.dma_start(cc_in[:], cs_row[:])
            nc.gpsimd.collective_compute(
                "AllReduce",
                mybir.AluOpType.add,
                replica_groups=[[0, 1, 2, 3], [4, 5, 6, 7]],
                ins=[cc_in[:]],
                outs=[cc_out[:]],
            )
            cs_sum = statp.tile([1, S], f32)
            nc.sync.dma_start(cs_sum[:], cc_out[:])
            # 1/cs = exp(-ln(cs - PAD_ROWS)); pad rows added exp(0)=1 each.
            # (vector.reciprocal is slow; ACT ln/exp is accurate to ~2e-5)
            negpad = statp.tile([1, 1], f32)
            nc.vector.memset(negpad[:], -float(PAD_ROWS))
            nc.scalar.activation(
                cs_row[:], cs_sum[:], AF.Ln, bias=negpad[:]
            )
            nc.scalar.activation(cs_row[:], cs_row[:], AF.Exp, scale=-1.0)
            cinv_d = dramp.tile([1, S], f32)
            nc.sync.dma_start(cinv_d[:], cs_row[:])
            crep = featp.tile([128, S], f32)
            nc.sync.dma_start(crep[:], cinv_d[:].partition_broadcast(128))

            # ================= sweep 2: conf =================
            for i in range(N_LTILES):
                g = gp.tile([128, S], f32, tag="gtile")
                for off, w in S2_CHUNKS:
                    ps = simps.tile([128, 1024], f32, tag="simps")
                    for k in range(2):
                        for so in range(0, w, 512):
                            sw = min(512, w - so)
                            nc.tensor.matmul(
                                ps[:, so:so + sw],
                                sf0[:, k * LM + i * 128: k * LM + (i + 1) * 128],
                                sf1[:, k * S + off + so: k * S + off + so + sw],
                                start=(k == 0),
                                stop=(k == 1),
                            )
                    nc.scalar.activation(
                        g[:, off:off + w], ps[:, :w], AF.Exp,
                        scale=2.0 * SCALE, bias=neg_lnrs[:, i:i + 1],
                    )
                nc.vector.tensor_tensor(
                    g[:], g[:], crep[:], op=mybir.AluOpType.mult
                )
                nc.sync.dma_start(conf_out[i * 128:(i + 1) * 128, :], g[:])

    nc.compile()
    return nc


_NC_CACHE = None


def _get_nc():
    global _NC_CACHE
    if _NC_CACHE is None:
        _NC_CACHE = _build()
    return _NC_CACHE


def _run(f0, f1):
    """f0, f1: [N_BATCH, 4800, 256] float32. Returns conf [N_BATCH, L, S]."""
    from concourse import bass_utils

    in_maps = []
    for core in range(N_CORES):
        b, j = divmod(core, 4)
        st = CHUNK_STARTS[j]
        sl = f0[b, st:st + LM, :]          # [<=1280, 256]
        if sl.shape[0] < LM:
            sl = np.concatenate(
                [sl, np.zeros((LM - sl.shape[0], C), np.float32)], axis=0
            )
        in_maps.append({
            "f0T_my": np.ascontiguousarray(sl.T),          # [256, 1280]
            "f1T_full": np.ascontiguousarray(f1[b].T),     # [256, 4800]
            "onesw": np.ones((128, 128), np.float32),
        })

    nc = _get_nc()
    res = bass_utils.run_bass_kernel_spmd(
        nc, in_maps, core_ids=list(range(N_CORES))
    )
    conf = np.empty((N_BATCH, L, S), np.float32)
    for core in range(N_CORES):
        b, j = divmod(core, 4)
        st = CHUNK_STARTS[j]
        n = min(LM, L - st)
        conf[b, st:st + n, :] = res.results[core]["conf_out"][:n, :]
    return conf


def _interior(n, b):
    a = np.arange(n)
    return (a >= b) & (a < n - b)


def _exact_mask(conf, h0, w0, h1, w1):
    """Exact numpy fallback for mask/matched_conf (never hit for randn
    inputs: conf stays ~3 orders of magnitude under THRESHOLD)."""
    N = conf.shape[0]
    mask = conf > THRESHOLD
    m5 = mask.reshape(N, h0, w0, h1, w1)
    valid = (
        _interior(h0, MARGIN)[:, None, None, None]
        & _interior(w0, MARGIN)[None, :, None, None]
        & _interior(h1, MARGIN)[None, None, :, None]
        & _interior(w1, MARGIN)[None, None, None, :]
    )
    m5 = m5 & valid[None]
    mask = m5.reshape(N, L, S)
    mutual = (conf == conf.max(axis=2, keepdims=True)) & (
        conf == conf.max(axis=1, keepdims=True)
    )
    mask = mask & mutual
    matched = np.where(mask, conf, 0.0).astype(np.float32)
    return mask, matched


def kernel(feature0, feature1, h0, w0, h1, w1):
    f0 = np.ascontiguousarray(np.asarray(feature0), dtype=np.float32)
    f1 = np.ascontiguousarray(np.asarray(feature1), dtype=np.float32)
    h0, w0, h1, w1 = int(h0), int(w0), int(h1), int(w1)

    conf = _run(f0, f1)

    if conf.max() > 0.95 * THRESHOLD:
        mask, matched = _exact_mask(conf, h0, w0, h1, w1)
    else:
        mask = np.zeros(conf.shape, dtype=bool)
        matched = np.zeros(conf.shape, dtype=np.float32)
    return conf, mask, matched


# revision 13
# speedup vs baseline: 1.0916x; 1.0916x over previous
"""Trainium2 Bass kernel for CoarseMatching (dual-softmax feature matching).

conf = softmax(sim, axis=2) * softmax(sim, axis=1),  sim = f0 @ f1^T / (C*TEMP)

Sharding: the L dimension of feature0 is split across 4 cores per batch
(2 batches x 4 chunks = 8 cores). Each core computes a [1280, 4800] row-slab
of conf (core 3 of each group is zero-padded from 960 to 1280 rows).

Per core (single NEFF, SPMD):
  sweep 1 (S-thirds outer so column-sum PSUM accumulators fit in 8 banks):
    sim tiles via fp32r matmul, E=exp(sim) with fused accum_out row-sums (rs),
    column-sums via ones-matmul accumulated in PSUM (partial: this core's rows).
  AllReduce(add) of cs partials across the 4-core batch group. The 320
  zero-pad rows contribute exactly exp(0)=1 each, removed via a -320 bias.
  sweep 2: recompute sim, conf = exp(2*sim - ln rs) * (1/cs) broadcast, DMA out.

mask / matched_conf: with randn inputs conf stays ~3 orders of magnitude below
THRESHOLD=0.2, so mask is all-False and matched_conf all-zero; the host checks
max(conf) and falls back to an exact numpy path if that ever fails.
"""

import numpy as np

TEMP = 0.1
THRESHOLD = 0.2
MARGIN = 2
N_BATCH = 2
L = 4800          # h0*w0
S = 4800          # h1*w1
C = 256
N_CORES = 8
LM = 1280         # padded per-core L-slab rows
PAD_ROWS = 4 * LM - L   # 320 zero rows per batch group (all on core 3)
SCALE = 1.0 / (C * TEMP)
CHUNK_STARTS = [0, 1280, 2560, 3840]

# S-thirds for sweep-1 column-sum PSUM accumulators: 4800 = 3 * 1600,
# each third split [1024, 576].
THIRDS = [(b, [(0, 1024), (1024, 576)]) for b in (0, 1600, 3200)]
# full-S chunking for sweep 2
S2_CHUNKS = [(0, 1024), (1024, 1024), (2048, 1024), (3072, 1024), (4096, 704)]
N_LTILES = LM // 128  # 10


def _build():
    from concourse import bacc, tile, mybir

    nc = bacc.Bacc(
        "TRN2", target_bir_lowering=False, debug=False, num_devices=N_CORES
    )
    f32 = mybir.dt.float32
    f32r = mybir.dt.float32r
    AF = mybir.ActivationFunctionType

    f0T_my = nc.dram_tensor("f0T_my", [C, LM], f32r, kind="ExternalInput").ap()
    f1T_full = nc.dram_tensor("f1T_full", [C, S], f32r, kind="ExternalInput").ap()
    onesw = nc.dram_tensor("onesw", [128, 128], f32r, kind="ExternalInput").ap()
    conf_out = nc.dram_tensor("conf_out", [LM, S], f32, kind="ExternalOutput").ap()

    with tile.TileContext(nc) as tc:
        with (
            tc.tile_pool(name="feat", bufs=1) as featp,
            tc.tile_pool(name="ep", bufs=3) as ep,
            tc.tile_pool(name="gp", bufs=3) as gp,
            tc.tile_pool(name="stats", bufs=1) as statp,
            tc.tile_pool(name="simps", bufs=2, space="PSUM") as simps,
            tc.tile_pool(name="csps", bufs=2, space="PSUM") as csps,
            tc.tile_pool(name="dram", bufs=1, space="DRAM") as dramp,
        ):
            # ---- load features (C on partitions, 2 K-halves side by side)
            sf0 = featp.tile([128, 2 * LM], f32r)
            sf1 = featp.tile([128, 2 * S], f32r)
            for k in range(2):
                nc.sync.dma_start(
                    sf0[:, k * LM:(k + 1) * LM], f0T_my[k * 128:(k + 1) * 128, :]
                )
                nc.sync.dma_start(
                    sf1[:, k * S:(k + 1) * S], f1T_full[k * 128:(k + 1) * 128, :]
                )

            ones = statp.tile([128, 128], f32r)
            nc.sync.dma_start(ones[:], onesw)

            rs_parts = statp.tile([128, N_LTILES * 6], f32)
            cs_row = statp.tile([1, S], f32)

            # ================= sweep 1: stats =================
            pending_cs = []  # deferred ones-matmuls so PE never waits on ACT

            def flush_one_cs():
                if pending_cs:
                    args = pending_cs.pop(0)
                    for a in args:
                        nc.tensor.matmul(*a[0], **a[1])

            for third, (tbase, tchunks) in enumerate(THIRDS):
                cs_acc = [
                    csps.tile([128, w], f32, name=f"cs_{tbase}_{ci}", tag="csacc")
                    for ci, (off, w) in enumerate(tchunks)
                ]
                for i in range(N_LTILES):
                    for ci, (off, w) in enumerate(tchunks):
                        ps = simps.tile([128, 1024], f32, tag="simps")
                        for k in range(2):
                            for so in range(0, w, 512):
                                sw = min(512, w - so)
                                nc.tensor.matmul(
                                    ps[:, so:so + sw],
                                    sf0[:, k * LM + i * 128: k * LM + (i + 1) * 128],
                                    sf1[:, k * S + tbase + off + so:
                                        k * S + tbase + off + so + sw],
                                    start=(k == 0),
                                    stop=(k == 1),
                                )
                        e = ep.tile([128, 1024], f32r, tag="etile")
                        # column layout must match the (i, c) grouping of the
                        # tensor_reduce below: col = i*6 + (third*2 + ci)
                        part = i * 6 + third * 2 + ci
                        nc.scalar.activation(
                            e[:, :w], ps[:, :w], AF.Exp, scale=SCALE,
                            accum_out=rs_parts[:, part:part + 1],
                        )
                        mm_args = []
                        for so in range(0, w, 512):
                            sw = min(512, w - so)
                            mm_args.append((
                                (
                                    cs_acc[ci][:, so:so + sw],
                                    ones[:],
                                    e[:, so:so + sw],
                                ),
                                dict(start=(i == 0), stop=(i == N_LTILES - 1)),
                            ))
                        pending_cs.append(mm_args)
                        if len(pending_cs) > 1:
                            flush_one_cs()
                while pending_cs:
                    flush_one_cs()
                for ci, (off, w) in enumerate(tchunks):
                    nc.vector.tensor_copy(
                        cs_row[0:1, tbase + off:tbase + off + w],
                        cs_acc[ci][0:1, :w],
                    )

            # ---- rs -> -ln(rs)  [128, N_LTILES]
            rs = statp.tile([128, N_LTILES], f32)
            nc.vector.tensor_reduce(
                rs[:],
                rs_parts[:].rearrange("p (i c) -> p i c", c=6),
                axis=mybir.AxisListType.X,
                op=mybir.AluOpType.add,
            )
            neg_lnrs = statp.tile([128, N_LTILES], f32)
            nc.scalar.activation(neg_lnrs[:], rs[:], AF.Ln)
            nc.vector.tensor_scalar_mul(neg_lnrs[:], neg_lnrs[:], -1.0)

            # ---- AllReduce cs partials across the 4-core batch group
            cc_in = dramp.tile([1, S], f32)
            cc_out = dramp.tile([1, S], f32)
            nc.sync.dma_start(cc_in[:], cs_row[:])
            nc.gpsimd.collective_compute(
                "AllReduce",
                mybir.AluOpType.add,
                replica_groups=[[0, 1, 2, 3], [4, 5, 6, 7]],
                ins=[cc_in[:]],
                outs=[cc_out[:]],
            )
            cs_sum = statp.tile([1, S], f32)
            nc.sync.dma_start(cs_sum[:], cc_out[:])
            # 1/cs = exp(-ln(cs - PAD_ROWS)); pad rows added exp(0)=1 each.
            # (vector.reciprocal is slow; ACT ln/exp is accurate to ~2e-5)
            negpad = statp.tile([1, 1], f32)
            nc.vector.memset(negpad[:], -float(PAD_ROWS))
            nc.scalar.activation(
                cs_row[:], cs_sum[:], AF.Ln, bias=negpad[:]
            )
            nc.scalar.activation(cs_row[:], cs_row[:], AF.Exp, scale=-1.0)
            cinv_d = dramp.tile([1, S], f32)
            nc.sync.dma_start(cinv_d[:], cs_row[:])
            crep = featp.tile([128, S], f32)
            nc.sync.dma_start(crep[:], cinv_d[:].partition_broadcast(128))

            # ================= sweep 2: conf =================
            for i in range(N_LTILES):
                g = gp.tile([128, S], f32, tag="gtile")
                for off, w in S2_CHUNKS:
                    ps = simps.tile([128, 1024], f32, tag="simps")
                    for k in range(2):
                        for so in range(0, w, 512):
                            sw = min(512, w - so)
                            nc.tensor.matmul(
                                ps[:, so:so + sw],
                                sf0[:, k * LM + i * 128: k * LM + (i + 1) * 128],
                                sf1[:, k * S + off + so: k * S + off + so + sw],
                                start=(k == 0),
                                stop=(k == 1),
                            )
                    nc.scalar.activation(
                        g[:, off:off + w], ps[:, :w], AF.Exp,
                        scale=2.0 * SCALE, bias=neg_lnrs[:, i:i + 1],
                    )
                nc.vector.tensor_tensor(
                    g[:], g[:], crep[:], op=mybir.AluOpType.mult
                )
                nc.sync.dma_start(conf_out[i * 128:(i + 1) * 128, :], g[:])

    nc.compile()
    return nc


_NC_CACHE = None


def _get_nc():
    global _NC_CACHE
    if _NC_CACHE is None:
        _NC_CACHE = _build()
    return _NC_CACHE


LAST_EXEC_NS = None


def _run(f0, f1, trace=False):
    """f0, f1: [N_BATCH, 4800, 256] float32. Returns conf [N_BATCH, L, S]."""
    global LAST_EXEC_NS
    from concourse import bass_utils

    in_maps = []
    for core in range(N_CORES):
        b, j = divmod(core, 4)
        st = CHUNK_STARTS[j]
        sl = f0[b, st:st + LM, :]          # [<=1280, 256]
        if sl.shape[0] < LM:
            sl = np.concatenate(
                [sl, np.zeros((LM - sl.shape[0], C), np.float32)], axis=0
            )
        in_maps.append({
            "f0T_my": np.ascontiguousarray(sl.T),          # [256, 1280]
            "f1T_full": np.ascontiguousarray(f1[b].T),     # [256, 4800]
            "onesw": np.ones((128, 128), np.float32),
        })

    nc = _get_nc()
    res = bass_utils.run_bass_kernel_spmd(
        nc, in_maps, core_ids=list(range(N_CORES)), trace=trace
    )
    if res.exec_time_ns is not None:
        LAST_EXEC_NS = res.exec_time_ns
    conf = np.empty((N_BATCH, L, S), np.float32)
    for core in range(N_CORES):
        b, j = divmod(core, 4)
        st = CHUNK_STARTS[j]
        n = min(LM, L - st)
        conf[b, st:st + n, :] = res.results[core]["conf_out"][:n, :]
    return conf


def _interior(n, b):
    a = np.arange(n)
    return (a >= b) & (a < n - b)


def _exact_mask(conf, h0, w0, h1, w1):
    """Exact numpy fallback for mask/matched_conf (never hit for randn
    inputs: conf stays ~3 orders of magnitude under THRESHOLD)."""
    N = conf.shape[0]
    mask = conf > THRESHOLD
    m5 = mask.reshape(N, h0, w0, h1, w1)
    valid = (
        _interior(h0, MARGIN)[:, None, None, None]
        & _interior(w0, MARGIN)[None, :, None, None]
        & _interior(h1, MARGIN)[None, None, :, None]
        & _interior(w1, MARGIN)[None, None, None, :]
    )
    m5 = m5 & valid[None]
    mask = m5.reshape(N, L, S)
    mutual = (conf == conf.max(axis=2, keepdims=True)) & (
        conf == conf.max(axis=1, keepdims=True)
    )
    mask = mask & mutual
    matched = np.where(mask, conf, 0.0).astype(np.float32)
    return mask, matched


def kernel(feature0, feature1, h0, w0, h1, w1):
    f0 = np.ascontiguousarray(np.asarray(feature0), dtype=np.float32)
    f1 = np.ascontiguousarray(np.asarray(feature1), dtype=np.float32)
    h0, w0, h1, w1 = int(h0), int(w0), int(h1), int(w1)

    conf = _run(f0, f1)

    if conf.max() > 0.95 * THRESHOLD:
        mask, matched = _exact_mask(conf, h0, w0, h1, w1)
    else:
        mask = np.zeros(conf.shape, dtype=bool)
        matched = np.zeros(conf.shape, dtype=np.float32)
    return conf, mask, matched


# revision 41
# speedup vs baseline: 1.3515x; 1.2381x over previous
"""Trainium2 Bass kernel for CoarseMatching (dual-softmax feature matching).

conf = softmax(sim, axis=2) * softmax(sim, axis=1),  sim = f0 @ f1^T / (C*TEMP)
     = exp(sim)^2 / (rowsum(exp sim) * colsum(exp sim))       [max-free: |sim|<6]

Sharding: the L dimension of feature0 is split across 4 cores per batch
(2 batches x 4 chunks = 8 cores). Each core computes a [1280, 4800] row-slab
of conf (core 3 of each group is zero-padded from 960 to 1280 rows).

Per core (single NEFF, SPMD), phases interleaved tile-by-tile:
  phase B tile t: simT = f1_my[t] @ f0_full^T via fp32r matmul; one ACT
    Exp pass per PSUM chunk with fused accum_out row-sums = COMPLETE column
    sums cs for this core's 1280 S-columns (free dim covers all 4800 L).
  phase A tile i: sim = f0_my[i] @ f1_full^T; ACT Exp -> E tile (fp32) with
    accum_out row-sums rs (complete: free dim covers all 4800 S);
    G = (E * (1/rs)) * E in one DVE scalar_tensor_tensor (in-place).
  AllGather(4-core group) of cs [1280] -> [5120]; 1/cs via ACT Ln+Exp;
  broadcast-replicate to [128,4800]; conf = G * (1/cs) (DVE/GPSIMD), DMA out.

Zero-pad rows need no correction: pads only sit in f0T_my / f1T_my (lhsT),
producing garbage conf rows (trimmed on host) and cs entries for columns
4800:5120 (never read). The _full tensors are unpadded.

mask / matched_conf: with randn inputs conf stays ~3 orders of magnitude below
THRESHOLD=0.2, so mask is all-False and matched_conf all-zero; the host checks
max(conf) and falls back to an exact numpy path if that ever fails.
"""

import numpy as np

TEMP = 0.1
THRESHOLD = 0.2
MARGIN = 2
N_BATCH = 2
L = 4800          # h0*w0
S = 4800          # h1*w1
C = 256
N_CORES = 8
LM = 1280         # padded per-core slab rows
SCALE = 1.0 / (C * TEMP)
CHUNK_STARTS = [0, 1280, 2560, 3840]

# PSUM chunking of the 4800-wide free dim: [128,2048] tiles = 4 banks;
# bufs=2 -> exactly 8 banks.
CHUNKS = [(0, 2048), (2048, 2048), (4096, 704)]
NCH = len(CHUNKS)
N_LTILES = LM // 128  # 10
USE_BF16 = True       # bf16 E tiles / crep / output (DVE 2x mode, half DMA)
GPS_TT = () if USE_BF16 else (1, 3, 5, 7, 9)  # finals on GPSIMD (fp32 only)


def _build(single=False):
    """single=True: 1-core variant with the collective replaced by a DMA
    copy — used only for cost-model timing (TimelineSim), not execution."""
    from concourse import bacc, tile, mybir

    nc = bacc.Bacc(
        "TRN2", target_bir_lowering=False, debug=False,
        num_devices=(1 if single else N_CORES),
    )
    f32 = mybir.dt.float32
    f32r = mybir.dt.float32r
    et = mybir.dt.bfloat16 if USE_BF16 else f32
    AF = mybir.ActivationFunctionType

    f0T_my = nc.dram_tensor("f0T_my", [C, LM], f32r, kind="ExternalInput").ap()
    f1T_my = nc.dram_tensor("f1T_my", [C, LM], f32r, kind="ExternalInput").ap()
    f0T_full = nc.dram_tensor("f0T_full", [C, S], f32r, kind="ExternalInput").ap()
    f1T_full = nc.dram_tensor("f1T_full", [C, S], f32r, kind="ExternalInput").ap()
    conf_out = nc.dram_tensor("conf_out", [LM, S], et, kind="ExternalOutput").ap()

    with tile.TileContext(nc) as tc:
        with (
            tc.tile_pool(name="feat", bufs=1) as featp,
            tc.tile_pool(name="ep", bufs=(8 if USE_BF16 else 4)) as ep,
            tc.tile_pool(name="jp", bufs=2) as jp,
            tc.tile_pool(name="stats", bufs=1) as statp,
            tc.tile_pool(name="simps", bufs=2, space="PSUM") as simps,
            tc.tile_pool(name="dram", bufs=1, space="DRAM") as dramp,
        ):
            # ---- load features (C on partitions, 2 K-halves side by side).
            # Phase-B inputs (f1T_my, f0T_full) first so B's matmuls start
            # as early as possible; phase-A inputs stream in behind them.
            sf0m = featp.tile([128, 2 * LM], f32r)
            sf1m = featp.tile([128, 2 * LM], f32r)
            sf0f = featp.tile([128, 2 * S], f32r)
            sf1f = featp.tile([128, 2 * S], f32r)
            for k in range(2):
                nc.sync.dma_start(
                    sf1m[:, k * LM:(k + 1) * LM], f1T_my[k * 128:(k + 1) * 128, :])
            H = S // 2
            for h in range(2):
                for k in range(2):
                    nc.sync.dma_start(
                        sf0f[:, k * S + h * H: k * S + (h + 1) * H],
                        f0T_full[k * 128:(k + 1) * 128, h * H:(h + 1) * H])
            for k in range(2):
                nc.sync.dma_start(
                    sf0m[:, k * LM:(k + 1) * LM], f0T_my[k * 128:(k + 1) * 128, :])
            for h in range(2):
                for k in range(2):
                    nc.sync.dma_start(
                        sf1f[:, k * S + h * H: k * S + (h + 1) * H],
                        f1T_full[k * 128:(k + 1) * 128, h * H:(h + 1) * H])

            cs_parts = statp.tile([128, N_LTILES * NCH], f32)
            rs_parts = statp.tile([128, N_LTILES * NCH], f32)
            u = statp.tile([128, N_LTILES], f32)
            etiles = []

            def mm_chunk(ps, lhsT_src, ti, rhs_src, off, w):
                for k in range(2):
                    for so in range(off, off + w, 512):
                        sw = min(512, off + w - so)
                        nc.tensor.matmul(
                            ps[:, so - off:so - off + sw],
                            lhsT_src[:, k * LM + ti * 128: k * LM + ti * 128 + 128],
                            rhs_src[:, k * S + so: k * S + so + sw],
                            start=(k == 0),
                            stop=(k == 1),
                        )

            # ---- phase B first (everything cs/collective needs), then A.
            # phase B tile t: simT chunks; COMPLETE colsums via accum_out.
            for t in range(N_LTILES):
                for ci, (off, w) in enumerate(CHUNKS):
                    ps = simps.tile([128, 2048], f32, tag="simps")
                    mm_chunk(ps, sf1m, t, sf0f, off, w)
                    junk = jp.tile([128, 2048], mybir.dt.bfloat16, tag="junk")
                    nc.scalar.activation(
                        junk[:, :w], ps[:, :w], AF.Exp, scale=SCALE,
                        accum_out=cs_parts[:, t * NCH + ci: t * NCH + ci + 1],
                    )

            # ---- cs -> AllGather -> 1/cs replicated (runs as soon as B done)
            cs3 = cs_parts[:].rearrange("p (t c) -> p t c", c=NCH)
            cs_my = statp.tile([128, N_LTILES], f32)
            nc.vector.tensor_tensor(
                cs_my[:], cs3[:, :, 0], cs3[:, :, 1], op=mybir.AluOpType.add
            )
            nc.vector.tensor_tensor(
                cs_my[:], cs_my[:], cs3[:, :, 2], op=mybir.AluOpType.add
            )
            bounce = dramp.tile([LM, 1], f32)
            nc.sync.dma_start(
                bounce[:].rearrange("(t p) o -> p (t o)", p=128), cs_my[:]
            )
            gath = dramp.tile([4 * LM, 1], f32)
            if single:
                nc.sync.dma_start(gath[0:LM, :], bounce[:])
                nc.sync.dma_start(gath[LM:2 * LM, :], bounce[:])
                nc.sync.dma_start(gath[2 * LM:3 * LM, :], bounce[:])
                nc.sync.dma_start(gath[3 * LM:4 * LM, :], bounce[:])
            else:
                nc.gpsimd.collective_compute(
                    "AllGather",
                    mybir.AluOpType.bypass,
                    replica_groups=[[0, 1, 2, 3], [4, 5, 6, 7]],
                    ins=[bounce[:]],
                    outs=[gath[:]],
                )
            # 1/cs via ACT Ln+Exp on a [96,50] parallel-lane layout (0.7us
            # instead of 8.6us single-lane), then broadcast-replicate.
            cs_l = statp.tile([96, 50], f32)
            nc.sync.dma_start(
                cs_l[:], gath[0:S, :].rearrange("(p j) o -> p (j o)", p=96)
            )
            cinv = statp.tile([96, 50], et)
            nc.scalar.activation(cs_l[:], cs_l[:], AF.Ln)
            nc.scalar.activation(cinv[:], cs_l[:], AF.Exp, scale=-1.0)
            cinv_d = dramp.tile([1, S], et)
            nc.sync.dma_start(
                cinv_d[:].rearrange("o (p j) -> p (j o)", p=96), cinv[:]
            )
            crep = featp.tile([128, S], et)
            nc.sync.dma_start(crep[:], cinv_d[:].partition_broadcast(128))

            # ---- phase A: E tile + rowsums; square early (crep-independent);
            # final conf = (E^2 * 1/rs) * (1/cs) once crep lands; DMA out.
            for i in range(N_LTILES):
                e = ep.tile([128, S], et, tag="etile", name=f"e_{i}")
                for ci, (off, w) in enumerate(CHUNKS):
                    ps = simps.tile([128, 2048], f32, tag="simps")
                    mm_chunk(ps, sf0m, i, sf1f, off, w)
                    nc.scalar.activation(
                        e[:, off:off + w], ps[:, :w], AF.Exp, scale=SCALE,
                        accum_out=rs_parts[:, i * NCH + ci: i * NCH + ci + 1],
                    )
                # u_i = 1/rs_i (tiny DVE add + reciprocal)
                nc.vector.scalar_tensor_tensor(
                    u[:, i:i + 1],
                    rs_parts[:, i * NCH:i * NCH + 1],
                    rs_parts[:, i * NCH + 1:i * NCH + 2],
                    rs_parts[:, i * NCH + 2:i * NCH + 3],
                    op0=mybir.AluOpType.add, op1=mybir.AluOpType.add,
                )
                nc.vector.reciprocal(u[:, i:i + 1], u[:, i:i + 1])
                # G = E^2 * u via TT (2x bf16) + tensor_scalar (4x bf16);
                # both crep-independent so they run before the collective.
                nc.vector.tensor_tensor(
                    e[:], e[:], e[:], op=mybir.AluOpType.mult
                )
                nc.vector.tensor_scalar_mul(e[:], e[:], u[:, i:i + 1])
                # final: conf = G * (1/cs); alternate DVE/GPSIMD
                eng = nc.gpsimd if i in GPS_TT else nc.vector
                eng.tensor_tensor(e[:], e[:], crep[:], op=mybir.AluOpType.mult)
                nc.sync.dma_start(conf_out[i * 128:(i + 1) * 128, :], e[:])

    nc.compile()
    return nc


_NC_CACHE = None


def _get_nc():
    global _NC_CACHE
    if _NC_CACHE is None:
        _NC_CACHE = _build()
    return _NC_CACHE


LAST_EXEC_NS = None


def _run(f0, f1, trace=False):
    """f0, f1: [N_BATCH, 4800, 256] float32. Returns conf [N_BATCH, L, S]."""
    global LAST_EXEC_NS
    from concourse import bass_utils

    in_maps = []
    for core in range(N_CORES):
        b, j = divmod(core, 4)
        st = CHUNK_STARTS[j]

        def slab(f):
            sl = f[b, st:st + LM, :]
            if sl.shape[0] < LM:
                sl = np.concatenate(
                    [sl, np.zeros((LM - sl.shape[0], C), np.float32)], axis=0)
            return np.ascontiguousarray(sl.T)

        in_maps.append({
            "f0T_my": slab(f0),                            # [256, 1280]
            "f1T_my": slab(f1),                            # [256, 1280]
            "f0T_full": np.ascontiguousarray(f0[b].T),     # [256, 4800]
            "f1T_full": np.ascontiguousarray(f1[b].T),     # [256, 4800]
        })

    nc = _get_nc()
    res = bass_utils.run_bass_kernel_spmd(
        nc, in_maps, core_ids=list(range(N_CORES)), trace=trace
    )
    if res.exec_time_ns is not None:
        LAST_EXEC_NS = res.exec_time_ns
    conf = np.empty((N_BATCH, L, S), np.float32)
    for core in range(N_CORES):
        b, j = divmod(core, 4)
        st = CHUNK_STARTS[j]
        n = min(LM, L - st)
        conf[b, st:st + n, :] = res.results[core]["conf_out"][:n, :].astype(
            np.float32
        )
    return conf


def _interior(n, b):
    a = np.arange(n)
    return (a >= b) & (a < n - b)


def _exact_mask(conf, h0, w0, h1, w1):
    """Exact numpy fallback for mask/matched_conf (never hit for randn
    inputs: conf stays ~3 orders of magnitude under THRESHOLD)."""
    N = conf.shape[0]
    mask = conf > THRESHOLD
    m5 = mask.reshape(N, h0, w0, h1, w1)
    valid = (
        _interior(h0, MARGIN)[:, None, None, None]
        & _interior(w0, MARGIN)[None, :, None, None]
        & _interior(h1, MARGIN)[None, None, :, None]
        & _interior(w1, MARGIN)[None, None, None, :]
    )
    m5 = m5 & valid[None]
    mask = m5.reshape(N, L, S)
    mutual = (conf == conf.max(axis=2, keepdims=True)) & (
        conf == conf.max(axis=1, keepdims=True)
    )
    mask = mask & mutual
    matched = np.where(mask, conf, 0.0).astype(np.float32)
    return mask, matched


def kernel(feature0, feature1, h0, w0, h1, w1):
    f0 = np.ascontiguousarray(np.asarray(feature0), dtype=np.float32)
    f1 = np.ascontiguousarray(np.asarray(feature1), dtype=np.float32)
    h0, w0, h1, w1 = int(h0), int(w0), int(h1), int(w1)

    conf = _run(f0, f1)

    if conf.max() > 0.95 * THRESHOLD:
        mask, matched = _exact_mask(conf, h0, w0, h1, w1)
    else:
        mask = np.zeros(conf.shape, dtype=bool)
        matched = np.zeros(conf.shape, dtype=np.float32)
    return conf, mask, matched


# revision 47
# speedup vs baseline: 1.3764x; 1.0184x over previous
"""Trainium2 Bass kernel for CoarseMatching (dual-softmax feature matching).

conf = softmax(sim, axis=2) * softmax(sim, axis=1),  sim = f0 @ f1^T / (C*TEMP)
     = exp(sim)^2 / (rowsum(exp sim) * colsum(exp sim))       [max-free: |sim|<6]

Sharding: the L dimension of feature0 is split across 4 cores per batch
(2 batches x 4 chunks = 8 cores). Each core computes a [1280, 4800] row-slab
of conf (core 3 of each group is zero-padded from 960 to 1280 rows).

Per core (single NEFF, SPMD), phases interleaved tile-by-tile:
  phase B tile t: simT = f1_my[t] @ f0_full^T via fp32r matmul; one ACT
    Exp pass per PSUM chunk with fused accum_out row-sums = COMPLETE column
    sums cs for this core's 1280 S-columns (free dim covers all 4800 L).
  phase A tile i: sim = f0_my[i] @ f1_full^T; ACT Exp -> E tile (fp32) with
    accum_out row-sums rs (complete: free dim covers all 4800 S);
    G = (E * (1/rs)) * E in one DVE scalar_tensor_tensor (in-place).
  AllGather(4-core group) of cs [1280] -> [5120]; 1/cs via ACT Ln+Exp;
  broadcast-replicate to [128,4800]; conf = G * (1/cs) (DVE/GPSIMD), DMA out.

Zero-pad rows need no correction: pads only sit in f0T_my / f1T_my (lhsT),
producing garbage conf rows (trimmed on host) and cs entries for columns
4800:5120 (never read). The _full tensors are unpadded.

mask / matched_conf: with randn inputs conf stays ~3 orders of magnitude below
THRESHOLD=0.2, so mask is all-False and matched_conf all-zero; the host checks
max(conf) and falls back to an exact numpy path if that ever fails.
"""

import numpy as np

TEMP = 0.1
THRESHOLD = 0.2
MARGIN = 2
N_BATCH = 2
L = 4800          # h0*w0
S = 4800          # h1*w1
C = 256
N_CORES = 8
LM = 1280         # padded per-core slab rows
SCALE = 1.0 / (C * TEMP)
CHUNK_STARTS = [0, 1280, 2560, 3840]

# PSUM chunking of the 4800-wide free dim: [128,2048] tiles = 4 banks;
# bufs=2 -> exactly 8 banks.
CHUNKS = [(0, 2048), (2048, 2048), (4096, 704)]
NCH = len(CHUNKS)
N_LTILES = LM // 128  # 10
OUT_BF16 = True           # bf16 conf output (halves output DMA; +~2e-3 err)
GPS_TT = (1, 3, 5, 7, 9)  # tiles whose final multiply runs on GPSIMD


def _build(single=False):
    """single=True: 1-core variant with the collective replaced by a DMA
    copy — used only for cost-model timing (TimelineSim), not execution."""
    from concourse import bacc, tile, mybir

    nc = bacc.Bacc(
        "TRN2", target_bir_lowering=False, debug=False,
        num_devices=(1 if single else N_CORES),
    )
    f32 = mybir.dt.float32
    f32r = mybir.dt.float32r
    et = mybir.dt.bfloat16 if OUT_BF16 else f32
    AF = mybir.ActivationFunctionType

    f0T_my = nc.dram_tensor("f0T_my", [C, LM], f32r, kind="ExternalInput").ap()
    f1T_my = nc.dram_tensor("f1T_my", [C, LM], f32r, kind="ExternalInput").ap()
    f0T_full = nc.dram_tensor("f0T_full", [C, S], f32r, kind="ExternalInput").ap()
    f1T_full = nc.dram_tensor("f1T_full", [C, S], f32r, kind="ExternalInput").ap()
    conf_out = nc.dram_tensor("conf_out", [LM, S], et, kind="ExternalOutput").ap()

    with tile.TileContext(nc) as tc:
        with (
            tc.tile_pool(name="feat", bufs=1) as featp,
            tc.tile_pool(name="ep", bufs=3) as ep,
            tc.tile_pool(name="op", bufs=2) as outp,
            tc.tile_pool(name="jp", bufs=1) as jp,
            tc.tile_pool(name="stats", bufs=1) as statp,
            tc.tile_pool(name="simps", bufs=2, space="PSUM") as simps,
            tc.tile_pool(name="dram", bufs=1, space="DRAM") as dramp,
        ):
            # ---- load features (C on partitions, 2 K-halves side by side).
            # Phase-B inputs (f1T_my, f0T_full) first so B's matmuls start
            # as early as possible; phase-A inputs stream in behind them.
            sf0m = featp.tile([128, 2 * LM], f32r)
            sf1m = featp.tile([128, 2 * LM], f32r)
            sf0f = featp.tile([128, 2 * S], f32r)
            sf1f = featp.tile([128, 2 * S], f32r)
            for k in range(2):
                nc.sync.dma_start(
                    sf1m[:, k * LM:(k + 1) * LM], f1T_my[k * 128:(k + 1) * 128, :])
            H = S // 2
            for h in range(2):
                for k in range(2):
                    nc.sync.dma_start(
                        sf0f[:, k * S + h * H: k * S + (h + 1) * H],
                        f0T_full[k * 128:(k + 1) * 128, h * H:(h + 1) * H])
            for k in range(2):
                nc.sync.dma_start(
                    sf0m[:, k * LM:(k + 1) * LM], f0T_my[k * 128:(k + 1) * 128, :])
            for h in range(2):
                for k in range(2):
                    nc.sync.dma_start(
                        sf1f[:, k * S + h * H: k * S + (h + 1) * H],
                        f1T_full[k * 128:(k + 1) * 128, h * H:(h + 1) * H])

            cs_parts = statp.tile([128, N_LTILES * NCH], f32)
            rs_parts = statp.tile([128, N_LTILES * NCH], f32)
            u = statp.tile([128, N_LTILES], f32)
            etiles = []

            def mm_chunk(ps, lhsT_src, ti, rhs_src, off, w):
                for k in range(2):
                    for so in range(off, off + w, 512):
                        sw = min(512, off + w - so)
                        nc.tensor.matmul(
                            ps[:, so - off:so - off + sw],
                            lhsT_src[:, k * LM + ti * 128: k * LM + ti * 128 + 128],
                            rhs_src[:, k * S + so: k * S + so + sw],
                            start=(k == 0),
                            stop=(k == 1),
                        )

            # ---- phase B first (everything cs/collective needs), then A.
            # phase B tile t: simT chunks; COMPLETE colsums via accum_out.
            for t in range(N_LTILES):
                for ci, (off, w) in enumerate(CHUNKS):
                    ps = simps.tile([128, 2048], f32, tag="simps")
                    mm_chunk(ps, sf1m, t, sf0f, off, w)
                    junk = jp.tile([128, 2048], mybir.dt.bfloat16, tag="junk")
                    nc.scalar.activation(
                        junk[:, :w], ps[:, :w], AF.Exp, scale=SCALE,
                        accum_out=cs_parts[:, t * NCH + ci: t * NCH + ci + 1],
                    )

            # ---- cs -> AllGather -> 1/cs replicated (runs as soon as B done)
            cs3 = cs_parts[:].rearrange("p (t c) -> p t c", c=NCH)
            cs_my = statp.tile([128, N_LTILES], f32)
            nc.vector.tensor_tensor(
                cs_my[:], cs3[:, :, 0], cs3[:, :, 1], op=mybir.AluOpType.add
            )
            nc.vector.tensor_tensor(
                cs_my[:], cs_my[:], cs3[:, :, 2], op=mybir.AluOpType.add
            )
            bounce = dramp.tile([LM, 1], f32)
            nc.sync.dma_start(
                bounce[:].rearrange("(t p) o -> p (t o)", p=128), cs_my[:]
            )
            gath = dramp.tile([4 * LM, 1], f32)
            if single:
                nc.sync.dma_start(gath[0:LM, :], bounce[:])
                nc.sync.dma_start(gath[LM:2 * LM, :], bounce[:])
                nc.sync.dma_start(gath[2 * LM:3 * LM, :], bounce[:])
                nc.sync.dma_start(gath[3 * LM:4 * LM, :], bounce[:])
            else:
                nc.gpsimd.collective_compute(
                    "AllGather",
                    mybir.AluOpType.bypass,
                    replica_groups=[[0, 1, 2, 3], [4, 5, 6, 7]],
                    ins=[bounce[:]],
                    outs=[gath[:]],
                )
            # 1/cs via ACT Ln+Exp on a [96,50] parallel-lane layout (0.7us
            # instead of 8.6us single-lane), then broadcast-replicate.
            cs_l = statp.tile([96, 50], f32)
            nc.sync.dma_start(
                cs_l[:], gath[0:S, :].rearrange("(p j) o -> p (j o)", p=96)
            )
            cinv = statp.tile([96, 50], f32)
            nc.scalar.activation(cs_l[:], cs_l[:], AF.Ln)
            nc.scalar.activation(cinv[:], cs_l[:], AF.Exp, scale=-1.0)
            cinv_d = dramp.tile([1, S], f32)
            nc.sync.dma_start(
                cinv_d[:].rearrange("o (p j) -> p (j o)", p=96), cinv[:]
            )
            crep = featp.tile([128, S], f32)
            nc.sync.dma_start(crep[:], cinv_d[:].partition_broadcast(128))

            # ---- phase A: E tile + rowsums; square early (crep-independent);
            # final conf = (E^2 * 1/rs) * (1/cs) once crep lands; DMA out.
            for i in range(N_LTILES):
                e = ep.tile([128, S], f32, tag="etile", name=f"e_{i}")
                for ci, (off, w) in enumerate(CHUNKS):
                    ps = simps.tile([128, 2048], f32, tag="simps")
                    mm_chunk(ps, sf0m, i, sf1f, off, w)
                    nc.scalar.activation(
                        e[:, off:off + w], ps[:, :w], AF.Exp, scale=SCALE,
                        accum_out=rs_parts[:, i * NCH + ci: i * NCH + ci + 1],
                    )
                # u_i = 1/rs_i (tiny DVE add + reciprocal)
                nc.vector.scalar_tensor_tensor(
                    u[:, i:i + 1],
                    rs_parts[:, i * NCH:i * NCH + 1],
                    rs_parts[:, i * NCH + 1:i * NCH + 2],
                    rs_parts[:, i * NCH + 2:i * NCH + 3],
                    op0=mybir.AluOpType.add, op1=mybir.AluOpType.add,
                )
                nc.vector.reciprocal(u[:, i:i + 1], u[:, i:i + 1])
                # G = (E * u) * E in one fused STT (crep-independent, in-place)
                nc.vector.scalar_tensor_tensor(
                    e[:], e[:], u[:, i:i + 1], e[:],
                    op0=mybir.AluOpType.mult, op1=mybir.AluOpType.mult,
                )
                # final: conf = G * (1/cs) -> bf16 out; alternate DVE/GPSIMD
                o = outp.tile([128, S], et, tag="otile", name=f"o_{i}")
                eng = nc.gpsimd if i in GPS_TT else nc.vector
                eng.tensor_tensor(o[:], e[:], crep[:], op=mybir.AluOpType.mult)
                nc.sync.dma_start(conf_out[i * 128:(i + 1) * 128, :], o[:])

    nc.compile()
    return nc


_NC_CACHE = None


def _get_nc():
    global _NC_CACHE
    if _NC_CACHE is None:
        _NC_CACHE = _build()
    return _NC_CACHE


LAST_EXEC_NS = None


def _run(f0, f1, trace=False):
    """f0, f1: [N_BATCH, 4800, 256] float32. Returns conf [N_BATCH, L, S]."""
    global LAST_EXEC_NS
    from concourse import bass_utils

    in_maps = []
    for core in range(N_CORES):
        b, j = divmod(core, 4)
        st = CHUNK_STARTS[j]

        def slab(f):
            sl = f[b, st:st + LM, :]
            if sl.shape[0] < LM:
                sl = np.concatenate(
                    [sl, np.zeros((LM - sl.shape[0], C), np.float32)], axis=0)
            return np.ascontiguousarray(sl.T)

        in_maps.append({
            "f0T_my": slab(f0),                            # [256, 1280]
            "f1T_my": slab(f1),                            # [256, 1280]
            "f0T_full": np.ascontiguousarray(f0[b].T),     # [256, 4800]
            "f1T_full": np.ascontiguousarray(f1[b].T),     # [256, 4800]
        })

    nc = _get_nc()
    res = bass_utils.run_bass_kernel_spmd(
        nc, in_maps, core_ids=list(range(N_CORES)), trace=trace
    )
    if res.exec_time_ns is not None:
        LAST_EXEC_NS = res.exec_time_ns
    conf = np.empty((N_BATCH, L, S), np.float32)
    for core in range(N_CORES):
        b, j = divmod(core, 4)
        st = CHUNK_STARTS[j]
        n = min(LM, L - st)
        conf[b, st:st + n, :] = res.results[core]["conf_out"][:n, :].astype(
            np.float32
        )
    return conf


def _interior(n, b):
    a = np.arange(n)
    return (a >= b) & (a < n - b)


def _exact_mask(conf, h0, w0, h1, w1):
    """Exact numpy fallback for mask/matched_conf (never hit for randn
    inputs: conf stays ~3 orders of magnitude under THRESHOLD)."""
    N = conf.shape[0]
    mask = conf > THRESHOLD
    m5 = mask.reshape(N, h0, w0, h1, w1)
    valid = (
        _interior(h0, MARGIN)[:, None, None, None]
        & _interior(w0, MARGIN)[None, :, None, None]
        & _interior(h1, MARGIN)[None, None, :, None]
        & _interior(w1, MARGIN)[None, None, None, :]
    )
    m5 = m5 & valid[None]
    mask = m5.reshape(N, L, S)
    mutual = (conf == conf.max(axis=2, keepdims=True)) & (
        conf == conf.max(axis=1, keepdims=True)
    )
    mask = mask & mutual
    matched = np.where(mask, conf, 0.0).astype(np.float32)
    return mask, matched


def kernel(feature0, feature1, h0, w0, h1, w1):
    f0 = np.ascontiguousarray(np.asarray(feature0), dtype=np.float32)
    f1 = np.ascontiguousarray(np.asarray(feature1), dtype=np.float32)
    h0, w0, h1, w1 = int(h0), int(w0), int(h1), int(w1)

    conf = _run(f0, f1)

    if conf.max() > 0.95 * THRESHOLD:
        mask, matched = _exact_mask(conf, h0, w0, h1, w1)
    else:
        mask = np.zeros(conf.shape, dtype=bool)
        matched = np.zeros(conf.shape, dtype=np.float32)
    return conf, mask, matched


# revision 50
# speedup vs baseline: 35092.3724x; 25495.5875x over previous
"""Trainium2 Bass kernel for CoarseMatching (dual-softmax feature matching).

conf = softmax(sim, axis=2) * softmax(sim, axis=1),  sim = f0 @ f1^T / (C*TEMP)
     = exp(sim)^2 / (rowsum(exp sim) * colsum(exp sim))       [max-free: |sim|<6]

Sharding: the L dimension of feature0 is split across 4 cores per batch
(2 batches x 4 chunks = 8 cores). Each core computes a [1280, 4800] row-slab
of conf (core 3 of each group is zero-padded from 960 to 1280 rows).

Per core (single NEFF, SPMD), phases interleaved tile-by-tile:
  phase B tile t: simT = f1_my[t] @ f0_full^T via fp32r matmul; one ACT
    Exp pass per PSUM chunk with fused accum_out row-sums = COMPLETE column
    sums cs for this core's 1280 S-columns (free dim covers all 4800 L).
  phase A tile i: sim = f0_my[i] @ f1_full^T; ACT Exp -> E tile (fp32) with
    accum_out row-sums rs (complete: free dim covers all 4800 S);
    G = (E * (1/rs)) * E in one DVE scalar_tensor_tensor (in-place).
  AllGather(4-core group) of cs [1280] -> [5120]; 1/cs via ACT Ln+Exp;
  broadcast-replicate to [128,4800]; conf = G * (1/cs) (DVE/GPSIMD), DMA out.

Zero-pad rows need no correction: pads only sit in f0T_my / f1T_my (lhsT),
producing garbage conf rows (trimmed on host) and cs entries for columns
4800:5120 (never read). The _full tensors are unpadded.

mask / matched_conf: with randn inputs conf stays ~3 orders of magnitude below
THRESHOLD=0.2, so mask is all-False and matched_conf all-zero; the host checks
max(conf) and falls back to an exact numpy path if that ever fails.
"""

import numpy as np

TEMP = 0.1
THRESHOLD = 0.2
MARGIN = 2
N_BATCH = 2
L = 4800          # h0*w0
S = 4800          # h1*w1
C = 256
N_CORES = 8
LM = 1280         # padded per-core slab rows
SCALE = 1.0 / (C * TEMP)
CHUNK_STARTS = [0, 1280, 2560, 3840]

# PSUM chunking of the 4800-wide free dim: [128,2048] tiles = 4 banks;
# bufs=2 -> exactly 8 banks.
CHUNKS = [(0, 2048), (2048, 2048), (4096, 704)]
NCH = len(CHUNKS)
N_LTILES = LM // 128  # 10
OUT_BF16 = True           # bf16 conf output (halves output DMA; +~2e-3 err)
GPS_TT = (2, 3, 5, 6, 7, 9)  # tiles whose final multiply runs on GPSIMD


def _build(single=False):
    """single=True: 1-core variant with the collective replaced by a DMA
    copy — used only for cost-model timing (TimelineSim), not execution."""
    from concourse import bacc, tile, mybir

    nc = bacc.Bacc(
        "TRN2", target_bir_lowering=False, debug=False,
        num_devices=(1 if single else N_CORES),
    )
    f32 = mybir.dt.float32
    f32r = mybir.dt.float32r
    et = mybir.dt.bfloat16 if OUT_BF16 else f32
    AF = mybir.ActivationFunctionType

    f0T_my = nc.dram_tensor("f0T_my", [C, LM], f32r, kind="ExternalInput").ap()
    f1T_my = nc.dram_tensor("f1T_my", [C, LM], f32r, kind="ExternalInput").ap()
    f0T_full = nc.dram_tensor("f0T_full", [C, S], f32r, kind="ExternalInput").ap()
    f1T_full = nc.dram_tensor("f1T_full", [C, S], f32r, kind="ExternalInput").ap()
    conf_out = nc.dram_tensor("conf_out", [LM, S], et, kind="ExternalOutput").ap()

    with tile.TileContext(nc) as tc:
        with (
            tc.tile_pool(name="feat", bufs=1) as featp,
            tc.tile_pool(name="ep", bufs=3) as ep,
            tc.tile_pool(name="op", bufs=2) as outp,
            tc.tile_pool(name="jp", bufs=1) as jp,
            tc.tile_pool(name="stats", bufs=1) as statp,
            tc.tile_pool(name="simps", bufs=2, space="PSUM") as simps,
            tc.tile_pool(name="dram", bufs=1, space="DRAM") as dramp,
        ):
            # ---- load features (C on partitions, 2 K-halves side by side).
            # Phase-B inputs (f1T_my, f0T_full) first so B's matmuls start
            # as early as possible; phase-A inputs stream in behind them.
            sf0m = featp.tile([128, 2 * LM], f32r)
            sf1m = featp.tile([128, 2 * LM], f32r)
            sf0f = featp.tile([128, 2 * S], f32r)
            sf1f = featp.tile([128, 2 * S], f32r)
            for k in range(2):
                nc.sync.dma_start(
                    sf1m[:, k * LM:(k + 1) * LM], f1T_my[k * 128:(k + 1) * 128, :])
            H = S // 2
            for h in range(2):
                for k in range(2):
                    nc.sync.dma_start(
                        sf0f[:, k * S + h * H: k * S + (h + 1) * H],
                        f0T_full[k * 128:(k + 1) * 128, h * H:(h + 1) * H])
            for k in range(2):
                nc.sync.dma_start(
                    sf0m[:, k * LM:(k + 1) * LM], f0T_my[k * 128:(k + 1) * 128, :])
            for h in range(2):
                for k in range(2):
                    nc.sync.dma_start(
                        sf1f[:, k * S + h * H: k * S + (h + 1) * H],
                        f1T_full[k * 128:(k + 1) * 128, h * H:(h + 1) * H])

            cs_parts = statp.tile([128, N_LTILES * NCH], f32)
            rs_parts = statp.tile([128, N_LTILES * NCH], f32)
            u = statp.tile([128, N_LTILES], f32)
            etiles = []

            def mm_chunk(ps, lhsT_src, ti, rhs_src, off, w):
                for k in range(2):
                    for so in range(off, off + w, 512):
                        sw = min(512, off + w - so)
                        nc.tensor.matmul(
                            ps[:, so - off:so - off + sw],
                            lhsT_src[:, k * LM + ti * 128: k * LM + ti * 128 + 128],
                            rhs_src[:, k * S + so: k * S + so + sw],
                            start=(k == 0),
                            stop=(k == 1),
                        )

            # ---- phase B first (everything cs/collective needs), then A.
            # phase B tile t: simT chunks; COMPLETE colsums via accum_out.
            for t in range(N_LTILES):
                for ci, (off, w) in enumerate(CHUNKS):
                    ps = simps.tile([128, 2048], f32, tag="simps")
                    mm_chunk(ps, sf1m, t, sf0f, off, w)
                    junk = jp.tile([128, 2048], mybir.dt.bfloat16, tag="junk")
                    nc.scalar.activation(
                        junk[:, :w], ps[:, :w], AF.Exp, scale=SCALE,
                        accum_out=cs_parts[:, t * NCH + ci: t * NCH + ci + 1],
                    )

            # ---- cs -> AllGather -> 1/cs replicated (runs as soon as B done)
            cs3 = cs_parts[:].rearrange("p (t c) -> p t c", c=NCH)
            cs_my = statp.tile([128, N_LTILES], f32)
            nc.vector.tensor_tensor(
                cs_my[:], cs3[:, :, 0], cs3[:, :, 1], op=mybir.AluOpType.add
            )
            nc.vector.tensor_tensor(
                cs_my[:], cs_my[:], cs3[:, :, 2], op=mybir.AluOpType.add
            )
            bounce = dramp.tile([LM, 1], f32)
            nc.sync.dma_start(
                bounce[:].rearrange("(t p) o -> p (t o)", p=128), cs_my[:]
            )
            gath = dramp.tile([4 * LM, 1], f32)
            if single:
                nc.sync.dma_start(gath[0:LM, :], bounce[:])
                nc.sync.dma_start(gath[LM:2 * LM, :], bounce[:])
                nc.sync.dma_start(gath[2 * LM:3 * LM, :], bounce[:])
                nc.sync.dma_start(gath[3 * LM:4 * LM, :], bounce[:])
            else:
                nc.gpsimd.collective_compute(
                    "AllGather",
                    mybir.AluOpType.bypass,
                    replica_groups=[[0, 1, 2, 3], [4, 5, 6, 7]],
                    ins=[bounce[:]],
                    outs=[gath[:]],
                )
            # 1/cs via ACT Ln+Exp on a [96,50] parallel-lane layout (0.7us
            # instead of 8.6us single-lane), then broadcast-replicate.
            cs_l = statp.tile([96, 50], f32)
            nc.sync.dma_start(
                cs_l[:], gath[0:S, :].rearrange("(p j) o -> p (j o)", p=96)
            )
            cinv = statp.tile([96, 50], f32)
            nc.scalar.activation(cs_l[:], cs_l[:], AF.Ln)
            nc.scalar.activation(cinv[:], cs_l[:], AF.Exp, scale=-1.0)
            cinv_d = dramp.tile([1, S], f32)
            nc.sync.dma_start(
                cinv_d[:].rearrange("o (p j) -> p (j o)", p=96), cinv[:]
            )
            crep = featp.tile([128, S], f32)
            nc.sync.dma_start(crep[:], cinv_d[:].partition_broadcast(128))

            # ---- phase A: E tile + rowsums; square early (crep-independent);
            # final conf = (E^2 * 1/rs) * (1/cs) once crep lands; DMA out.
            for i in range(N_LTILES):
                e = ep.tile([128, S], f32, tag="etile", name=f"e_{i}")
                for ci, (off, w) in enumerate(CHUNKS):
                    ps = simps.tile([128, 2048], f32, tag="simps")
                    mm_chunk(ps, sf0m, i, sf1f, off, w)
                    nc.scalar.activation(
                        e[:, off:off + w], ps[:, :w], AF.Exp, scale=SCALE,
                        accum_out=rs_parts[:, i * NCH + ci: i * NCH + ci + 1],
                    )
                # u_i = 1/rs_i (tiny DVE add + reciprocal)
                nc.vector.scalar_tensor_tensor(
                    u[:, i:i + 1],
                    rs_parts[:, i * NCH:i * NCH + 1],
                    rs_parts[:, i * NCH + 1:i * NCH + 2],
                    rs_parts[:, i * NCH + 2:i * NCH + 3],
                    op0=mybir.AluOpType.add, op1=mybir.AluOpType.add,
                )
                nc.vector.reciprocal(u[:, i:i + 1], u[:, i:i + 1])
                # G = (E * u) * E in one fused STT (crep-independent, in-place)
                nc.vector.scalar_tensor_tensor(
                    e[:], e[:], u[:, i:i + 1], e[:],
                    op0=mybir.AluOpType.mult, op1=mybir.AluOpType.mult,
                )
                # final: conf = G * (1/cs) -> bf16 out; alternate DVE/GPSIMD
                o = outp.tile([128, S], et, tag="otile", name=f"o_{i}")
                eng = nc.gpsimd if i in GPS_TT else nc.vector
                eng.tensor_tensor(o[:], e[:], crep[:], op=mybir.AluOpType.mult)
                nc.sync.dma_start(conf_out[i * 128:(i + 1) * 128, :], o[:])

    nc.compile()
    return nc


_NC_CACHE = None


def _get_nc():
    global _NC_CACHE
    if _NC_CACHE is None:
        _NC_CACHE = _build()
    return _NC_CACHE


LAST_EXEC_NS = None


def _run(f0, f1, trace=False):
    """f0, f1: [N_BATCH, 4800, 256] float32. Returns conf [N_BATCH, L, S]."""
    global LAST_EXEC_NS
    from concourse import bass_utils

    in_maps = []
    for core in range(N_CORES):
        b, j = divmod(core, 4)
        st = CHUNK_STARTS[j]

        def slab(f):
            sl = f[b, st:st + LM, :]
            if sl.shape[0] < LM:
                sl = np.concatenate(
                    [sl, np.zeros((LM - sl.shape[0], C), np.float32)], axis=0)
            return np.ascontiguousarray(sl.T)

        in_maps.append({
            "f0T_my": slab(f0),                            # [256, 1280]
            "f1T_my": slab(f1),                            # [256, 1280]
            "f0T_full": np.ascontiguousarray(f0[b].T),     # [256, 4800]
            "f1T_full": np.ascontiguousarray(f1[b].T),     # [256, 4800]
        })

    nc = _get_nc()
    res = bass_utils.run_bass_kernel_spmd(
        nc, in_maps, core_ids=list(range(N_CORES)), trace=trace
    )
    if res.exec_time_ns is not None:
        LAST_EXEC_NS = res.exec_time_ns
    conf = np.empty((N_BATCH, L, S), np.float32)
    for core in range(N_CORES):
        b, j = divmod(core, 4)
        st = CHUNK_STARTS[j]
        n = min(LM, L - st)
        conf[b, st:st + n, :] = res.results[core]["conf_out"][:n, :].astype(
            np.float32
        )
    return conf


def _interior(n, b):
    a = np.arange(n)
    return (a >= b) & (a < n - b)


def _exact_mask(conf, h0, w0, h1, w1):
    """Exact numpy fallback for mask/matched_conf (never hit for randn
    inputs: conf stays ~3 orders of magnitude under THRESHOLD)."""
    N = conf.shape[0]
    mask = conf > THRESHOLD
    m5 = mask.reshape(N, h0, w0, h1, w1)
    valid = (
        _interior(h0, MARGIN)[:, None, None, None]
        & _interior(w0, MARGIN)[None, :, None, None]
        & _interior(h1, MARGIN)[None, None, :, None]
        & _interior(w1, MARGIN)[None, None, None, :]
    )
    m5 = m5 & valid[None]
    mask = m5.reshape(N, L, S)
    mutual = (conf == conf.max(axis=2, keepdims=True)) & (
        conf == conf.max(axis=1, keepdims=True)
    )
    mask = mask & mutual
    matched = np.where(mask, conf, 0.0).astype(np.float32)
    return mask, matched


def kernel(feature0, feature1, h0, w0, h1, w1):
    f0 = np.ascontiguousarray(np.asarray(feature0), dtype=np.float32)
    f1 = np.ascontiguousarray(np.asarray(feature1), dtype=np.float32)
    h0, w0, h1, w1 = int(h0), int(w0), int(h1), int(w1)

    conf = _run(f0, f1)

    if conf.max() > 0.95 * THRESHOLD:
        mask, matched = _exact_mask(conf, h0, w0, h1, w1)
    else:
        mask = np.zeros(conf.shape, dtype=bool)
        matched = np.zeros(conf.shape, dtype=np.float32)
    return conf, mask, matched
